# revision 3
# baseline (speedup 1.0000x reference)
"""BertSelfAttention on 8 TRN2 NeuronCores (Bass/Tile).

Sharding: tensor-parallel over heads. Core c computes heads 2c, 2c+1
(output dims 128c : 128c+128). Each core receives the full hidden states
(pre-transposed and cast to bf16 on the host) plus its slice of the
Q/K/V projection weights, and produces its [B, S, 128] slice of the
context output; the host concatenates slices along the feature axis.

Per-core pipeline (B=4, S=2048, H=1024, NH=16, HD=64; 2 heads/core):
  1. proj:  QT,KT [128, S] (head dim on partitions) and V [S, 128]
            (seq on partitions) via PE matmuls over 8 contraction chunks.
  2. attn:  for each 512-wide query chunk:
              for each 128-wide key block:
                ST[k,q] = KT^T @ QT  (two heads packed into PE row halves,
                                      contraction = head_dim = 64)
                P = exp(ST/8 + mask) on ScalarE (psum -> sbuf bf16)
                OT[d,q] += [V | 1]^T-style matmul: lhsT=[V,ones] (M=65)
                           accumulating both context and row-sums l.
              normalize: transpose OT -> [q, d], multiply by 1/l.
The bv bias is folded in on the host (rows of softmax sum to one, so
ctx(V + bv) = ctx(V) + bv exactly).
"""

import numpy as np
import ml_dtypes

import concourse.bass as bass
import concourse.mybir as mybir
import concourse.tile as tile
from concourse import bass_utils
from concourse.masks import make_identity
from concourse.vector_clock import ScopedClock

B, S, H, NH, HD = 4, 2048, 1024, 16, 64
N_CORES = 8
DH = H // N_CORES          # 128 output dims per core (2 heads)
P = 128
QC = 512                   # query chunk (psum bank width in fp32)
NQC = S // QC              # 4
NKB = S // P               # 16 key blocks
NHC = H // P               # 8 contraction chunks for the projections
BF16 = mybir.dt.bfloat16
F32 = mybir.dt.float32


def _split_multi_waits(nc):
    # walrus in this container accepts at most ONE sync wait per
    # instruction; hoist extra waits onto preceding same-engine NOPs.
    n = 0
    for bb in nc.m.functions[0].blocks:
        new_insts = []
        for inst in bb.instructions:
            si = inst.sync_info
            if si is not None and si.on_wait and len(si.on_wait) > 1:
                waits = list(si.on_wait)
                for w in waits[:-1]:
                    n += 1
                    new_insts.append(
                        mybir.InstNoOp(
                            name=f"waitsplit_{n}",
                            engine=inst.engine,
                            bass_nofuse=True,
                            sync_info=mybir.SyncInfo(on_wait=[w], on_update=[]),
                        )
                    )
                si.on_wait = waits[-1:]
            new_insts.append(inst)
        bb.instructions[:] = new_insts


def build_bass():
    nc = bass.Bass("TRN2", target_bir_lowering=False, debug=False)
    xt = nc.dram_tensor("xt", [B, H, S], BF16, kind="ExternalInput").ap()
    wqt = nc.dram_tensor("wqt", [H, DH], BF16, kind="ExternalInput").ap()
    wkt = nc.dram_tensor("wkt", [H, DH], BF16, kind="ExternalInput").ap()
    wvt = nc.dram_tensor("wvt", [H, DH], BF16, kind="ExternalInput").ap()
    bqv = nc.dram_tensor("bqv", [DH], F32, kind="ExternalInput").ap()
    bkv = nc.dram_tensor("bkv", [DH], F32, kind="ExternalInput").ap()
    mask = nc.dram_tensor("mask", [B, S], F32, kind="ExternalInput").ap()
    out = nc.dram_tensor("out", [B, S, DH], F32, kind="ExternalOutput").ap()

    with tile.TileContext(nc) as tc:
        from contextlib import ExitStack

        with ExitStack() as ctx:
            consts = ctx.enter_context(tc.tile_pool(name="consts", bufs=1))
            xt_pool = ctx.enter_context(tc.tile_pool(name="xt", bufs=2))
            qkt_pool = ctx.enter_context(tc.tile_pool(name="qkt", bufs=2))
            von_pool = ctx.enter_context(tc.tile_pool(name="von", bufs=2))
            ex_pool = ctx.enter_context(tc.tile_pool(name="ex", bufs=3))
            s01_pool = ctx.enter_context(tc.tile_pool(name="s01", bufs=2))
            rb_pool = ctx.enter_context(tc.tile_pool(name="rb", bufs=4))
            osb_pool = ctx.enter_context(tc.tile_pool(name="osb", bufs=2))
            mask_pool = ctx.enter_context(tc.tile_pool(name="maskp", bufs=2))
            ps_misc = ctx.enter_context(tc.tile_pool(name="ps_misc", bufs=2, space="PSUM"))
            ps_st = ctx.enter_context(tc.tile_pool(name="ps_st", bufs=2, space="PSUM"))
            ps_ot = ctx.enter_context(tc.tile_pool(name="ps_ot", bufs=2, space="PSUM"))

            # constants
            wq_sb = consts.tile([P, NHC, DH], BF16, name="wq_sb")
            wk_sb = consts.tile([P, NHC, DH], BF16, name="wk_sb")
            wv_sb = consts.tile([P, NHC, DH], BF16, name="wv_sb")
            nc.sync.dma_start(wq_sb[:], wqt.rearrange("(hc p) d -> p hc d", p=P))
            nc.sync.dma_start(wk_sb[:], wkt.rearrange("(hc p) d -> p hc d", p=P))
            nc.sync.dma_start(wv_sb[:], wvt.rearrange("(hc p) d -> p hc d", p=P))
            bq_sb = consts.tile([P, 1], F32, name="bq_sb")
            bk_sb = consts.tile([P, 1], F32, name="bk_sb")
            nc.sync.dma_start(bq_sb[:], bqv[:, None])
            nc.sync.dma_start(bk_sb[:], bkv[:, None])
            ident = consts.tile([P, P], F32, name="ident")
            make_identity(nc, ident[:])

            for b in range(B):
                xt_b = xt_pool.tile([P, NHC, S], BF16, name="xt_b")
                nc.sync.dma_start(xt_b[:], xt[b].rearrange("(hc p) s -> p hc s", p=P))
                mask_b = mask_pool.tile([P, NKB], F32, name="mask_b")
                nc.sync.dma_start(mask_b[:], mask[b].rearrange("(kb p) -> p kb", p=P))

                # ---- projections ----
                qt = qkt_pool.tile([P, S], BF16, name="qt")
                kt = qkt_pool.tile([P, S], BF16, name="kt")
                for qc in range(NQC):
                    pq = ps_misc.tile([P, QC], F32, name="pq", tag="misc")
                    for h in range(NHC):
                        nc.tensor.matmul(
                            pq[:],
                            lhsT=wq_sb[:, h, :],
                            rhs=xt_b[:, h, qc * QC:(qc + 1) * QC],
                            start=(h == 0),
                            stop=(h == NHC - 1),
                        )
                    nc.vector.tensor_tensor(
                        qt[:, qc * QC:(qc + 1) * QC],
                        pq[:],
                        bq_sb[:].to_broadcast((P, QC)),
                        mybir.AluOpType.add,
                    )
                    pk = ps_misc.tile([P, QC], F32, name="pk", tag="misc")
                    for h in range(NHC):
                        nc.tensor.matmul(
                            pk[:],
                            lhsT=wk_sb[:, h, :],
                            rhs=xt_b[:, h, qc * QC:(qc + 1) * QC],
                            start=(h == 0),
                            stop=(h == NHC - 1),
                        )
                    nc.vector.tensor_tensor(
                        kt[:, qc * QC:(qc + 1) * QC],
                        pk[:],
                        bk_sb[:].to_broadcast((P, QC)),
                        mybir.AluOpType.add,
                    )

                # V in [s, d] layout with a ones column per head:
                # von[:, kb, 0:64] = V head0, [:, kb, 64] = 1,
                # von[:, kb, 65:129] = V head1, [:, kb, 129] = 1
                von = von_pool.tile([P, NKB, 2 * (HD + 1)], BF16, name="von")
                nc.vector.memset(von[:, :, HD:HD + 1], 1.0)
                nc.vector.memset(von[:, :, 2 * HD + 1:2 * HD + 2], 1.0)
                for kb in range(NKB):
                    pv = ps_misc.tile([P, P], F32, name="pv", tag="misc")
                    for h in range(NHC):
                        nc.tensor.matmul(
                            pv[:],
                            lhsT=xt_b[:, h, kb * P:(kb + 1) * P],
                            rhs=wv_sb[:, h, :],
                            start=(h == 0),
                            stop=(h == NHC - 1),
                        )
                    nc.vector.tensor_copy(von[:, kb, 0:HD], pv[:, 0:HD])
                    nc.vector.tensor_copy(von[:, kb, HD + 1:2 * HD + 1], pv[:, HD:2 * HD])

                # ---- attention ----
                for qc in range(NQC):
                    qsl = slice(qc * QC, (qc + 1) * QC)
                    ot0 = ps_ot.tile([P, QC], F32, name="ot0", tag="ot")
                    ot1 = ps_ot.tile([P, QC], F32, name="ot1", tag="ot")
                    for kb in range(NKB):
                        stp = ps_st.tile([P, 2 * QC], F32, name="stp")
                        nc.tensor.matmul(
                            stp[:, 0:QC],
                            lhsT=kt[0:HD, kb * P:(kb + 1) * P],
                            rhs=qt[0:HD, qsl],
                            start=True,
                            stop=True,
                        )
                        nc.tensor.matmul(
                            stp[:, QC:2 * QC],
                            lhsT=kt[HD:2 * HD, kb * P:(kb + 1) * P],
                            rhs=qt[HD:2 * HD, qsl],
                            start=True,
                            stop=True,
                        )
                        ex = ex_pool.tile([P, 2 * QC], BF16, name="ex")
                        nc.scalar.activation(
                            ex[:],
                            stp[:],
                            mybir.ActivationFunctionType.Exp,
                            bias=mask_b[:, kb:kb + 1],
                            scale=1.0 / np.sqrt(HD),
                        )
                        nc.tensor.matmul(
                            ot0[0:HD + 1, :],
                            lhsT=von[:, kb, 0:HD + 1],
                            rhs=ex[:, 0:QC],
                            start=(kb == 0),
                            stop=(kb == NKB - 1),
                        )
                        nc.tensor.matmul(
                            ot1[0:HD + 1, :],
                            lhsT=von[:, kb, HD + 1:2 * HD + 2],
                            rhs=ex[:, QC:2 * QC],
                            start=(kb == 0),
                            stop=(kb == NKB - 1),
                        )

                    # normalize + transpose to [q, d] and store
                    s0 = s01_pool.tile([HD + 1, QC], F32, name="s0", tag="s01")
                    s1 = s01_pool.tile([HD + 1, QC], F32, name="s1", tag="s01")
                    nc.vector.tensor_copy(s0[:], ot0[0:HD + 1, :])
                    nc.vector.tensor_copy(s1[:], ot1[0:HD + 1, :])
                    osb = osb_pool.tile([P, QC // P, DH], F32, name="osb")
                    for j in range(QC // P):
                        jsl = slice(j * P, (j + 1) * P)
                        o2t0 = ps_misc.tile([P, HD + 1], F32, name="o2t0", tag="misc")
                        nc.tensor.transpose(
                            o2t0[:], s0[:, jsl], ident[0:HD + 1, 0:HD + 1]
                        )
                        o2t1 = ps_misc.tile([P, HD + 1], F32, name="o2t1", tag="misc")
                        nc.tensor.transpose(
                            o2t1[:], s1[:, jsl], ident[0:HD + 1, 0:HD + 1]
                        )
                        rb0 = rb_pool.tile([P, 1], F32, name="rb0", tag="rb")
                        rb1 = rb_pool.tile([P, 1], F32, name="rb1", tag="rb")
                        nc.vector.reciprocal(rb0[:], o2t0[:, HD:HD + 1])
                        nc.vector.reciprocal(rb1[:], o2t1[:, HD:HD + 1])
                        nc.vector.tensor_scalar_mul(osb[:, j, 0:HD], o2t0[:, 0:HD], rb0[:])
                        nc.vector.tensor_scalar_mul(osb[:, j, HD:2 * HD], o2t1[:, 0:HD], rb1[:])
                    nc.sync.dma_start(
                        out[b].rearrange("(a p) d -> p a d", p=P)[
                            :, qc * (QC // P):(qc + 1) * (QC // P), :
                        ],
                        osb[:],
                    )
    _split_multi_waits(nc)
    return nc


def host_prep(hidden_states, attention_mask, Wq, bq, Wk, bk, Wv, bv):
    xt_np = np.ascontiguousarray(
        np.asarray(hidden_states).transpose(0, 2, 1)
    ).astype(ml_dtypes.bfloat16)
    mask_np = np.ascontiguousarray(
        np.asarray(attention_mask).reshape(B, S)
    ).astype(np.float32)
    in_maps = []
    for c in range(N_CORES):
        dsl = slice(c * DH, (c + 1) * DH)
        in_maps.append(
            {
                "xt": xt_np,
                "wqt": np.ascontiguousarray(np.asarray(Wq)[dsl, :].T).astype(ml_dtypes.bfloat16),
                "wkt": np.ascontiguousarray(np.asarray(Wk)[dsl, :].T).astype(ml_dtypes.bfloat16),
                "wvt": np.ascontiguousarray(np.asarray(Wv)[dsl, :].T).astype(ml_dtypes.bfloat16),
                "bqv": np.ascontiguousarray(np.asarray(bq)[dsl]).astype(np.float32),
                "bkv": np.ascontiguousarray(np.asarray(bk)[dsl]).astype(np.float32),
                "mask": mask_np,
            }
        )
    return in_maps


def gather(results, bv):
    out = np.empty((B, S, H), np.float32)
    for c in range(N_CORES):
        out[:, :, c * DH:(c + 1) * DH] = results[c]["out"]
    # bv folded on the host: softmax rows sum to 1, so ctx(V+bv)=ctx(V)+bv
    out += np.asarray(bv).astype(np.float32)[None, None, :]
    return out


def kernel(hidden_states, attention_mask, Wq, bq, Wk, bk, Wv, bv):
    in_maps = host_prep(hidden_states, attention_mask, Wq, bq, Wk, bk, Wv, bv)
    nc = build_bass()
    res = bass_utils.run_bass_kernel_spmd(nc, in_maps, core_ids=list(range(N_CORES)))
    return gather(res.results, bv)


# revision 5
# speedup vs baseline: 164.1316x; 164.1316x over previous
"""BertSelfAttention on 8 TRN2 NeuronCores (Bass/Tile).

Sharding: tensor-parallel over heads. Core c computes heads 2c, 2c+1
(output dims 128c : 128c+128). Each core receives the full hidden states
(pre-transposed and cast to bf16 on the host) plus its slice of the
Q/K/V projection weights, and produces its [B, S, 128] slice of the
context output; the host concatenates slices along the feature axis.

Per-core pipeline (B=4, S=2048, H=1024, NH=16, HD=64; 2 heads/core):
  1. proj:  QT,KT [128, S] (head dim on partitions) and V [S, 128]
            (seq on partitions) via PE matmuls over 8 contraction chunks.
  2. attn:  for each 512-wide query chunk:
              for each 128-wide key block:
                ST[k,q] = KT^T @ QT  (two heads packed into PE row halves,
                                      contraction = head_dim = 64)
                P = exp(ST/8 + mask) on ScalarE (psum -> sbuf bf16)
                OT[d,q] += [V | 1]^T-style matmul: lhsT=[V,ones] (M=65)
                           accumulating both context and row-sums l.
              normalize: transpose OT -> [q, d], multiply by 1/l.
The bv bias is folded in on the host (rows of softmax sum to one, so
ctx(V + bv) = ctx(V) + bv exactly).
"""

import numpy as np
import ml_dtypes

import concourse.bass as bass
import concourse.mybir as mybir
import concourse.tile as tile
from concourse import bass_utils
from concourse.masks import make_identity
from concourse.vector_clock import ScopedClock

B, S, H, NH, HD = 4, 2048, 1024, 16, 64
N_CORES = 8
DH = H // N_CORES          # 128 output dims per core (2 heads)
P = 128
QC = 512                   # query chunk (psum bank width in fp32)
NQC = S // QC              # 4
NKB = S // P               # 16 key blocks
NHC = H // P               # 8 contraction chunks for the projections
BF16 = mybir.dt.bfloat16
F32 = mybir.dt.float32


def _split_multi_waits(nc):
    # walrus in this container accepts at most ONE sync wait per
    # instruction; hoist extra waits onto preceding same-engine NOPs.
    n = 0
    for bb in nc.m.functions[0].blocks:
        new_insts = []
        for inst in bb.instructions:
            si = inst.sync_info
            if si is not None and si.on_wait and len(si.on_wait) > 1:
                waits = list(si.on_wait)
                for w in waits[:-1]:
                    n += 1
                    new_insts.append(
                        mybir.InstNoOp(
                            name=f"waitsplit_{n}",
                            engine=inst.engine,
                            bass_nofuse=True,
                            sync_info=mybir.SyncInfo(on_wait=[w], on_update=[]),
                        )
                    )
                si.on_wait = waits[-1:]
            new_insts.append(inst)
        bb.instructions[:] = new_insts


def build_bass(reps=1):
    nc = bass.Bass("TRN2", target_bir_lowering=False, debug=False)
    xt = nc.dram_tensor("xt", [B, H, S], BF16, kind="ExternalInput").ap()
    wqt = nc.dram_tensor("wqt", [H, DH], BF16, kind="ExternalInput").ap()
    wkt = nc.dram_tensor("wkt", [H, DH], BF16, kind="ExternalInput").ap()
    wvt = nc.dram_tensor("wvt", [H, DH], BF16, kind="ExternalInput").ap()
    bqv = nc.dram_tensor("bqv", [DH], F32, kind="ExternalInput").ap()
    bkv = nc.dram_tensor("bkv", [DH], F32, kind="ExternalInput").ap()
    mask = nc.dram_tensor("mask", [B, S], F32, kind="ExternalInput").ap()
    out = nc.dram_tensor("out", [B, S, DH], F32, kind="ExternalOutput").ap()

    with tile.TileContext(nc) as tc:
        from contextlib import ExitStack

        with ExitStack() as ctx:
            consts = ctx.enter_context(tc.tile_pool(name="consts", bufs=1))
            xt_pool = ctx.enter_context(tc.tile_pool(name="xt", bufs=2))
            qkt_pool = ctx.enter_context(tc.tile_pool(name="qkt", bufs=2))
            von_pool = ctx.enter_context(tc.tile_pool(name="von", bufs=2))
            ex_pool = ctx.enter_context(tc.tile_pool(name="ex", bufs=3))
            s01_pool = ctx.enter_context(tc.tile_pool(name="s01", bufs=2))
            rb_pool = ctx.enter_context(tc.tile_pool(name="rb", bufs=4))
            osb_pool = ctx.enter_context(tc.tile_pool(name="osb", bufs=2))
            mask_pool = ctx.enter_context(tc.tile_pool(name="maskp", bufs=2))
            ps_misc = ctx.enter_context(tc.tile_pool(name="ps_misc", bufs=2, space="PSUM"))
            ps_st = ctx.enter_context(tc.tile_pool(name="ps_st", bufs=2, space="PSUM"))
            ps_ot = ctx.enter_context(tc.tile_pool(name="ps_ot", bufs=2, space="PSUM"))

            # constants
            wq_sb = consts.tile([P, NHC, DH], BF16, name="wq_sb")
            wk_sb = consts.tile([P, NHC, DH], BF16, name="wk_sb")
            wv_sb = consts.tile([P, NHC, DH], BF16, name="wv_sb")
            nc.sync.dma_start(wq_sb[:], wqt.rearrange("(hc p) d -> p hc d", p=P))
            nc.sync.dma_start(wk_sb[:], wkt.rearrange("(hc p) d -> p hc d", p=P))
            nc.sync.dma_start(wv_sb[:], wvt.rearrange("(hc p) d -> p hc d", p=P))
            bq_sb = consts.tile([P, 1], F32, name="bq_sb")
            bk_sb = consts.tile([P, 1], F32, name="bk_sb")
            nc.sync.dma_start(bq_sb[:], bqv[:, None])
            nc.sync.dma_start(bk_sb[:], bkv[:, None])
            ident = consts.tile([P, P], F32, name="ident")
            make_identity(nc, ident[:])

            for b in [b for _ in range(reps) for b in range(B)]:
                xt_b = xt_pool.tile([P, NHC, S], BF16, name="xt_b")
                nc.sync.dma_start(xt_b[:], xt[b].rearrange("(hc p) s -> p hc s", p=P))
                mask_b = mask_pool.tile([P, NKB], F32, name="mask_b")
                nc.sync.dma_start(mask_b[:], mask[b].rearrange("(kb p) -> p kb", p=P))

                # ---- projections ----
                qt = qkt_pool.tile([P, S], BF16, name="qt")
                kt = qkt_pool.tile([P, S], BF16, name="kt")
                for qc in range(NQC):
                    pq = ps_misc.tile([P, QC], F32, name="pq", tag="misc")
                    for h in range(NHC):
                        nc.tensor.matmul(
                            pq[:],
                            lhsT=wq_sb[:, h, :],
                            rhs=xt_b[:, h, qc * QC:(qc + 1) * QC],
                            start=(h == 0),
                            stop=(h == NHC - 1),
                        )
                    nc.vector.tensor_tensor(
                        qt[:, qc * QC:(qc + 1) * QC],
                        pq[:],
                        bq_sb[:].to_broadcast((P, QC)),
                        mybir.AluOpType.add,
                    )
                    pk = ps_misc.tile([P, QC], F32, name="pk", tag="misc")
                    for h in range(NHC):
                        nc.tensor.matmul(
                            pk[:],
                            lhsT=wk_sb[:, h, :],
                            rhs=xt_b[:, h, qc * QC:(qc + 1) * QC],
                            start=(h == 0),
                            stop=(h == NHC - 1),
                        )
                    nc.vector.tensor_tensor(
                        kt[:, qc * QC:(qc + 1) * QC],
                        pk[:],
                        bk_sb[:].to_broadcast((P, QC)),
                        mybir.AluOpType.add,
                    )

                # V in [s, d] layout with a ones column per head:
                # von[:, kb, 0:64] = V head0, [:, kb, 64] = 1,
                # von[:, kb, 65:129] = V head1, [:, kb, 129] = 1
                von = von_pool.tile([P, NKB, 2 * (HD + 1)], BF16, name="von")
                nc.vector.memset(von[:, :, HD:HD + 1], 1.0)
                nc.vector.memset(von[:, :, 2 * HD + 1:2 * HD + 2], 1.0)
                for kb in range(NKB):
                    pv = ps_misc.tile([P, P], F32, name="pv", tag="misc")
                    for h in range(NHC):
                        nc.tensor.matmul(
                            pv[:],
                            lhsT=xt_b[:, h, kb * P:(kb + 1) * P],
                            rhs=wv_sb[:, h, :],
                            start=(h == 0),
                            stop=(h == NHC - 1),
                        )
                    nc.vector.tensor_copy(von[:, kb, 0:HD], pv[:, 0:HD])
                    nc.vector.tensor_copy(von[:, kb, HD + 1:2 * HD + 1], pv[:, HD:2 * HD])

                # ---- attention ----
                for qc in range(NQC):
                    qsl = slice(qc * QC, (qc + 1) * QC)
                    ot0 = ps_ot.tile([P, QC], F32, name="ot0", tag="ot")
                    ot1 = ps_ot.tile([P, QC], F32, name="ot1", tag="ot")
                    for kb in range(NKB):
                        stp = ps_st.tile([P, 2 * QC], F32, name="stp")
                        nc.tensor.matmul(
                            stp[:, 0:QC],
                            lhsT=kt[0:HD, kb * P:(kb + 1) * P],
                            rhs=qt[0:HD, qsl],
                            start=True,
                            stop=True,
                        )
                        nc.tensor.matmul(
                            stp[:, QC:2 * QC],
                            lhsT=kt[HD:2 * HD, kb * P:(kb + 1) * P],
                            rhs=qt[HD:2 * HD, qsl],
                            start=True,
                            stop=True,
                        )
                        ex = ex_pool.tile([P, 2 * QC], BF16, name="ex")
                        nc.scalar.activation(
                            ex[:],
                            stp[:],
                            mybir.ActivationFunctionType.Exp,
                            bias=mask_b[:, kb:kb + 1],
                            scale=1.0 / np.sqrt(HD),
                        )
                        nc.tensor.matmul(
                            ot0[0:HD + 1, :],
                            lhsT=von[:, kb, 0:HD + 1],
                            rhs=ex[:, 0:QC],
                            start=(kb == 0),
                            stop=(kb == NKB - 1),
                        )
                        nc.tensor.matmul(
                            ot1[0:HD + 1, :],
                            lhsT=von[:, kb, HD + 1:2 * HD + 2],
                            rhs=ex[:, QC:2 * QC],
                            start=(kb == 0),
                            stop=(kb == NKB - 1),
                        )

                    # normalize + transpose to [q, d] and store
                    s0 = s01_pool.tile([HD + 1, QC], F32, name="s0", tag="s01")
                    s1 = s01_pool.tile([HD + 1, QC], F32, name="s1", tag="s01")
                    nc.vector.tensor_copy(s0[:], ot0[0:HD + 1, :])
                    nc.vector.tensor_copy(s1[:], ot1[0:HD + 1, :])
                    osb = osb_pool.tile([P, QC // P, DH], F32, name="osb")
                    for j in range(QC // P):
                        jsl = slice(j * P, (j + 1) * P)
                        o2t0 = ps_misc.tile([P, HD + 1], F32, name="o2t0", tag="misc")
                        nc.tensor.transpose(
                            o2t0[:], s0[:, jsl], ident[0:HD + 1, 0:HD + 1]
                        )
                        o2t1 = ps_misc.tile([P, HD + 1], F32, name="o2t1", tag="misc")
                        nc.tensor.transpose(
                            o2t1[:], s1[:, jsl], ident[0:HD + 1, 0:HD + 1]
                        )
                        rb0 = rb_pool.tile([P, 1], F32, name="rb0", tag="rb")
                        rb1 = rb_pool.tile([P, 1], F32, name="rb1", tag="rb")
                        nc.vector.reciprocal(rb0[:], o2t0[:, HD:HD + 1])
                        nc.vector.reciprocal(rb1[:], o2t1[:, HD:HD + 1])
                        nc.vector.tensor_scalar_mul(osb[:, j, 0:HD], o2t0[:, 0:HD], rb0[:])
                        nc.vector.tensor_scalar_mul(osb[:, j, HD:2 * HD], o2t1[:, 0:HD], rb1[:])
                    nc.sync.dma_start(
                        out[b].rearrange("(a p) d -> p a d", p=P)[
                            :, qc * (QC // P):(qc + 1) * (QC // P), :
                        ],
                        osb[:],
                    )
    _split_multi_waits(nc)
    return nc


def host_prep(hidden_states, attention_mask, Wq, bq, Wk, bk, Wv, bv):
    xt_np = np.ascontiguousarray(
        np.asarray(hidden_states).transpose(0, 2, 1)
    ).astype(ml_dtypes.bfloat16)
    mask_np = np.ascontiguousarray(
        np.asarray(attention_mask).reshape(B, S)
    ).astype(np.float32)
    in_maps = []
    for c in range(N_CORES):
        dsl = slice(c * DH, (c + 1) * DH)
        in_maps.append(
            {
                "xt": xt_np,
                "wqt": np.ascontiguousarray(np.asarray(Wq)[dsl, :].T).astype(ml_dtypes.bfloat16),
                "wkt": np.ascontiguousarray(np.asarray(Wk)[dsl, :].T).astype(ml_dtypes.bfloat16),
                "wvt": np.ascontiguousarray(np.asarray(Wv)[dsl, :].T).astype(ml_dtypes.bfloat16),
                "bqv": np.ascontiguousarray(np.asarray(bq)[dsl]).astype(np.float32),
                "bkv": np.ascontiguousarray(np.asarray(bk)[dsl]).astype(np.float32),
                "mask": mask_np,
            }
        )
    return in_maps


def gather(results, bv):
    out = np.empty((B, S, H), np.float32)
    for c in range(N_CORES):
        out[:, :, c * DH:(c + 1) * DH] = results[c]["out"]
    # bv folded on the host: softmax rows sum to 1, so ctx(V+bv)=ctx(V)+bv
    out += np.asarray(bv).astype(np.float32)[None, None, :]
    return out


def kernel(hidden_states, attention_mask, Wq, bq, Wk, bk, Wv, bv):
    in_maps = host_prep(hidden_states, attention_mask, Wq, bq, Wk, bk, Wv, bv)
    nc = build_bass()
    res = bass_utils.run_bass_kernel_spmd(nc, in_maps, core_ids=list(range(N_CORES)))
    return gather(res.results, bv)
